# revision 38
# baseline (speedup 1.0000x reference)
"""Multi-head attention (B=2, S=2048, E=1024, H=16, causal) on 8 TRN2 cores.

Sharding: core c -> batch b = c//4, head group g = c%4 (4 heads each).
Each core computes QKV projection for its heads, causal flash-style
attention (no-max softmax, denominator via ones-column appended to V),
and a partial output projection against a 256-row slice of W_proj.
Host sums the 4 partial projections per batch (the "all-reduce") and
stacks the 2 batches.

All matmul operands are float32r (TF32-like single-pass PE matmul, fp32
accumulation in PSUM). Activation layouts are chosen so no on-device
transposes are needed: the host passes x[b].T per core.
"""
import sys

sys.path.insert(0, "/opt/trn_rl_repo")

import numpy as np

import concourse.bacc as bacc
import concourse.mybir as mybir
from concourse import tile
from concourse.bass_utils import run_bass_kernel_spmd

B, S, E, H, D = 2, 2048, 1024, 16, 64
SCALE = D ** -0.5
N_CORES = 8
HL = 4            # heads per core
GC = 256          # channel columns per core (HL * D)
F32 = mybir.dt.float32
F32R = mybir.dt.float32r

_CACHED_NC = None


def _build():
    nc = bacc.Bacc("TRN2", target_bir_lowering=False, debug=False,
                   num_devices=N_CORES)

    xT = nc.dram_tensor("xT", [E, S], F32R, kind="ExternalInput")
    w = nc.dram_tensor("w", [E, 3 * GC], F32R, kind="ExternalInput")
    wp = nc.dram_tensor("wp", [GC, E], F32R, kind="ExternalInput")
    mask = nc.dram_tensor("mask", [128, 128], F32R, kind="ExternalInput")
    y = nc.dram_tensor("y", [S, E], F32, kind="ExternalOutput")

    ET = E // 128     # 8 e-tiles
    NS = S // 512     # 4 s-chunks of 512
    NT = S // 128     # 16 s-tiles of 128

    with tile.TileContext(nc) as tc:
        with (
            tc.tile_pool(name="const", bufs=1) as cst,
            tc.tile_pool(name="acts", bufs=1) as acts,
            tc.tile_pool(name="expp", bufs=5) as expp,
            tc.tile_pool(name="small", bufs=2) as small,
            tc.tile_pool(name="ysb", bufs=3) as ysbp,
            tc.tile_pool(name="psS", bufs=2, space="PSUM") as psS,
            tc.tile_pool(name="psO", bufs=4, space="PSUM") as psO,
        ):
            # ---- constant loads -------------------------------------------
            xt = cst.tile([128, ET, S], F32R)          # x[b]^T  (e on partitions)
            wt = cst.tile([128, ET, 3 * GC], F32R)     # W_qkv slice (e on partitions)
            wpt = cst.tile([128, 2, E], F32R)          # W_proj slice (c on partitions)
            mk = cst.tile([128, 128], F32R)            # tril(128) causal mask

            # weights first, then x chunk 0, so QKV of chunk 0 starts ASAP;
            # remaining x chunks stream in behind it
            wr = w[:].rearrange("(t p) j -> p t j", p=128)
            xTr = xT[:].rearrange("(t p) s -> p t s", p=128)
            nc.sync.dma_start(wt[:, 0:4, 0:512], wr[:, 0:4, 0:512])
            nc.sync.dma_start(xt[:, 0:4, 0:512], xTr[:, 0:4, 0:512])
            nc.sync.dma_start(wt[:, 4:8, 0:512], wr[:, 4:8, 0:512])
            nc.sync.dma_start(xt[:, 4:8, 0:512], xTr[:, 4:8, 0:512])
            nc.sync.dma_start(wt[:, :, 512:768], wr[:, :, 512:768])
            nc.sync.dma_start(mk[:], mask[:])
            nc.sync.dma_start(wpt[:], wp[:].rearrange("(t p) e -> p t e", p=128))
            for sc in range(1, NS):
                nc.sync.dma_start(
                    xt[:, :, 512 * sc:512 * (sc + 1)], xTr[:, :, 512 * sc:512 * (sc + 1)]
                )

            # ---- activation buffers ---------------------------------------
            # qt/kt: [pair, j(128 part: head 2p on 0-63, head 2p+1 on 64-127), s]
            qt = acts.tile([128, 2, S], F32R)
            kt = acts.tile([128, 2, S], F32R)
            # v_aug: per s-tile, per head 65 cols (64 data + ones)
            vt = acts.tile([128, NT, HL * 65], F32R)
            # attention output^T, proj lhsT layout: c on partitions
            ot = acts.tile([128, 2, S], F32R)

            nc.vector.memset(vt[:].bitcast(F32), 1.0)
            warm = cst.tile([128, 16], F32)
            nc.scalar.activation(warm[:], vt[:, 0, 0:16].bitcast(F32),
                                 mybir.ActivationFunctionType.Exp)
            # dummy matmuls during the DMA head keep the PE HAM-warm so QKV
            # starts at full clock
            vflat = vt[:].rearrange("p a b -> p (a b)")
            for wi in range(22):
                pw = psS.tile([128, 1024], F32, tag="ps", name="pw")
                nc.tensor.matmul(pw[:, 0:512], vflat[:, 0:128], vflat[:, 512:1024],
                                 start=True, stop=True)

            # ---- QKV projection ------------------------------------------
            def qk_pair(sc, jt):
                s0 = 512 * sc
                for which, dest in ((0, qt), (1, kt)):
                    ps = psO.tile([128, 512], F32, tag="po", name="psqk")
                    for et in range(ET):
                        nc.tensor.matmul(
                            ps[:, 0:512],
                            wt[:, et, 256 * which + 128 * jt:256 * which + 128 * (jt + 1)],
                            xt[:, et, s0:s0 + 512],
                            start=(et == 0),
                            stop=(et == ET - 1),
                        )
                    nc.vector.tensor_copy(dest[:, jt, s0:s0 + 512], ps[:, 0:512])

            def v_chunk(sc, lo=0, hi=4):
                for st4 in range(lo, hi):
                    st = 4 * sc + st4
                    ps = psO.tile([128, 512], F32, tag="po", name="psv")
                    for et in range(ET):
                        nc.tensor.matmul(
                            ps[:, 0:256],
                            xt[:, et, 128 * st:128 * (st + 1)],
                            wt[:, et, 512:768],
                            start=(et == 0),
                            stop=(et == ET - 1),
                        )
                    nc.vector.tensor_copy(
                        vt[:, st].rearrange("p (h m) -> p h m", h=HL)[:, :, 0:64],
                        ps[:, 0:256].rearrange("p (h m) -> p h m", h=HL),
                    )

            # ---- attention for one (pair, q-chunk) ------------------------
            def attn_part(pr, jq, ik_lo, ik_hi, o_ab):
                s0 = 512 * jq
                nik = 4 * jq + 4
                for ik in range(ik_lo, ik_hi):
                    t = ik - 4 * jq
                    c0 = 128 * t if t > 0 else 0   # exact-causal column trim
                    ps = psS.tile([128, 1024], F32)
                    for ab in range(2):
                        p0 = 64 * ab
                        nc.tensor.matmul(
                            ps[:, 512 * ab + c0:512 * (ab + 1)],
                            kt[p0:p0 + 64, pr, 128 * ik:128 * (ik + 1)],
                            qt[p0:p0 + 64, pr, s0 + c0:s0 + 512],
                            start=True,
                            stop=True,
                            tile_position=(p0, 0),
                        )
                    e = expp.tile([128, 1024], F32R, tag="exps", name="exps")
                    e3 = e[:].rearrange("p (h n) -> p h n", h=2)[:, :, c0:512]
                    ps3 = ps[:].rearrange("p (h n) -> p h n", h=2)[:, :, c0:512]
                    nc.scalar.activation(e3, ps3, mybir.ActivationFunctionType.Exp,
                                         scale=float(SCALE))
                    if t >= 0:
                        # only the first 128 cols of the trimmed range are
                        # partially masked; the rest is fully unmasked
                        for ab in range(2):
                            nc.vector.tensor_mul(
                                e[:, 512 * ab + c0:512 * ab + c0 + 128],
                                e[:, 512 * ab + c0:512 * ab + c0 + 128],
                                mk[:],
                            )
                    for ab in range(2):
                        h = 2 * pr + ab
                        nc.tensor.matmul(
                            o_ab[ab][0:65, c0:512],
                            vt[:, ik, 65 * h:65 * h + 65],
                            e[:, 512 * ab + c0:512 * (ab + 1)],
                            start=(ik == 0),
                            stop=(ik == nik - 1),
                            skip_group_check=True,
                        )
            # normalize: out^T[d, s] = o[d, s] * (1 / o[64, s]).
            # Engines can shift partitions between in and out APs, so head
            # ab=1 writes ot partitions 64-127 directly.
            def attn_norm(pr, jq, o_ab):
                s0 = 512 * jq
                for ab in range(2):
                    o = o_ab[ab]
                    osb = small.tile([128, 512], F32R, tag="osb", name="osb")
                    nc.scalar.copy(osb[0:65, :], o[0:65, :])
                    # r row to physical partition 0 (partition_broadcast needs it)
                    rz = small.tile([128, 512], F32, tag="rz", name="rz")
                    nc.vector.tensor_copy(rz[0:1, :], osb[64:65, :].bitcast(F32))
                    rb = small.tile([128, 512], F32, tag="rb", name="rb")
                    nc.gpsimd.partition_broadcast(rb[0:64, :], rz[0:1, :])
                    rinv = small.tile([128, 512], F32, tag="rinv", name="rinv")
                    nc.vector.reciprocal_approx_fast(rinv[0:64, :], rb[0:64, :])
                    nc.vector.tensor_mul(ot[64 * ab:64 * ab + 64, pr, s0:s0 + 512],
                                         osb[0:64, :], rinv[0:64, :].bitcast(F32R))

            # ---- output projection for one s-chunk ------------------------
            def proj(jq, copy_eng=None):
                for st4 in range(4):
                    st = 4 * jq + st4
                    for nk in range(2):
                        py = psO.tile([128, 512], F32, tag="po", name="py")
                        for ct in range(2):
                            nc.tensor.matmul(
                                py[:],
                                ot[:, ct, 128 * st:128 * (st + 1)],
                                wpt[:, ct, 512 * nk:512 * (nk + 1)],
                                start=(ct == 0),
                                stop=(ct == 1),
                            )
                        ys = ysbp.tile([128, 512], F32)
                        if copy_eng == "scalar":
                            nc.scalar.copy(ys[:], py[:])
                        else:
                            nc.vector.tensor_copy(ys[:], py[:])
                        nc.sync.dma_start(
                            y[128 * st:128 * (st + 1), 512 * nk:512 * (nk + 1)], ys[:]
                        )

            # Interleave QKV chunks and proj between attention sub-phases:
            # attention is ACT(exp)-bound, so the PE queue gets dense
            # independent matmul work to stay HAM-warm, and proj trails one
            # sub-phase behind so its ot deps (incl. the head-B shift DMA)
            # are long complete when the PE reaches it.
            def attn_full(pr, jq, mids):
                """attention with qkv/proj work interleaved at explicit
                ik split points: mids = [(split_ik, fn), ...] ascending"""
                nik = 4 * jq + 4
                o_ab = [psO.tile([128, 512], F32, tag="po", name="o_ab")
                        for _ in range(2)]
                prev = 0
                for split, fn in mids:
                    attn_part(pr, jq, prev, split, o_ab)
                    fn()
                    prev = split
                attn_part(pr, jq, prev, nik, o_ab)
                attn_norm(pr, jq, o_ab)

            qk_pair(0, 0)
            v_chunk(0)
            attn_full(0, 0, [(2, lambda: qk_pair(0, 1))])
            attn_full(1, 0, [(2, lambda: qk_pair(1, 0)),
                             (2, lambda: qk_pair(1, 1))])
            attn_full(0, 1, [(3, lambda: v_chunk(1, 0, 2)),
                             (4, lambda: v_chunk(1, 2, 4))])
            proj(0)
            attn_full(1, 1, [(4, lambda: qk_pair(2, 0))])
            attn_full(0, 2, [(4, lambda: qk_pair(2, 1)),
                             (8, lambda: v_chunk(2))])
            proj(1)
            attn_full(1, 2, [(6, lambda: qk_pair(3, 0))])
            attn_full(0, 3, [(5, lambda: qk_pair(3, 1)),
                             (11, lambda: v_chunk(3))])
            proj(2)
            attn_full(1, 3, [])
            proj(3, copy_eng="scalar")

    nc.compile()
    return nc


def _get_nc():
    global _CACHED_NC
    if _CACHED_NC is None:
        _CACHED_NC = _build()
    return _CACHED_NC


def _diag_masks() -> np.ndarray:
    return np.ascontiguousarray(np.tril(np.ones((128, 128), dtype=np.float32)).T)


def _in_maps(x, W_qkv, W_proj):
    masks = _diag_masks()
    maps = []
    for c in range(N_CORES):
        b, g = divmod(c, 4)
        xT = np.ascontiguousarray(x[b].T.astype(np.float32))
        wq = W_qkv[:, GC * g:GC * (g + 1)]
        wk = W_qkv[:, E + GC * g:E + GC * (g + 1)]
        wv = W_qkv[:, 2 * E + GC * g:2 * E + GC * (g + 1)]
        w = np.ascontiguousarray(
            np.concatenate([wq, wk, wv], axis=1).astype(np.float32))
        wp = np.ascontiguousarray(W_proj[GC * g:GC * (g + 1), :].astype(np.float32))
        maps.append({"xT": xT, "w": w, "wp": wp, "mask": masks})
    return maps


def _run(x, W_qkv, W_proj, trace=False, **spmd_kwargs):
    nc = _get_nc()
    res = run_bass_kernel_spmd(nc, _in_maps(x, W_qkv, W_proj),
                               list(range(N_CORES)), trace=trace, **spmd_kwargs)
    out = np.zeros((B, S, E), dtype=np.float32)
    for c in range(N_CORES):
        out[c // 4] += res.results[c]["y"]
    return out, res


def kernel(x, attention_mask, W_qkv, W_proj):
    x = np.asarray(x, dtype=np.float32)
    W_qkv = np.asarray(W_qkv, dtype=np.float32)
    W_proj = np.asarray(W_proj, dtype=np.float32)
    out, _ = _run(x, W_qkv, W_proj, trace=False)
    return out


# revision 39
# speedup vs baseline: 1.0183x; 1.0183x over previous
"""Multi-head attention (B=2, S=2048, E=1024, H=16, causal) on 8 TRN2 cores.

Sharding: core c -> batch b = c//4, head group g = c%4 (4 heads each).
Each core computes QKV projection for its heads, causal flash-style
attention (no-max softmax, denominator via ones-column appended to V),
and a partial output projection against a 256-row slice of W_proj.
Host sums the 4 partial projections per batch (the "all-reduce") and
stacks the 2 batches.

All matmul operands are float32r (TF32-like single-pass PE matmul, fp32
accumulation in PSUM). Activation layouts are chosen so no on-device
transposes are needed: the host passes x[b].T per core.
"""
import sys

sys.path.insert(0, "/opt/trn_rl_repo")

import numpy as np

import concourse.bacc as bacc
import concourse.mybir as mybir
from concourse import tile
from concourse.bass_utils import run_bass_kernel_spmd

B, S, E, H, D = 2, 2048, 1024, 16, 64
SCALE = D ** -0.5
N_CORES = 8
HL = 4            # heads per core
GC = 256          # channel columns per core (HL * D)
F32 = mybir.dt.float32
F32R = mybir.dt.float32r

_CACHED_NC = None


def _build():
    nc = bacc.Bacc("TRN2", target_bir_lowering=False, debug=False,
                   num_devices=N_CORES)

    xT = nc.dram_tensor("xT", [E, S], F32R, kind="ExternalInput")
    w = nc.dram_tensor("w", [E, 3 * GC], F32R, kind="ExternalInput")
    wp = nc.dram_tensor("wp", [GC, E], F32R, kind="ExternalInput")
    mask = nc.dram_tensor("mask", [128, 128], F32R, kind="ExternalInput")
    y = nc.dram_tensor("y", [S, E], F32, kind="ExternalOutput")

    ET = E // 128     # 8 e-tiles
    NS = S // 512     # 4 s-chunks of 512
    NT = S // 128     # 16 s-tiles of 128

    with tile.TileContext(nc) as tc:
        with (
            tc.tile_pool(name="const", bufs=1) as cst,
            tc.tile_pool(name="acts", bufs=1) as acts,
            tc.tile_pool(name="expp", bufs=5) as expp,
            tc.tile_pool(name="small", bufs=2) as small,
            tc.tile_pool(name="ysb", bufs=3) as ysbp,
            tc.tile_pool(name="psS", bufs=2, space="PSUM") as psS,
            tc.tile_pool(name="psO", bufs=4, space="PSUM") as psO,
        ):
            # ---- constant loads -------------------------------------------
            xt = cst.tile([128, ET, S], F32R)          # x[b]^T  (e on partitions)
            wt = cst.tile([128, ET, 3 * GC], F32R)     # W_qkv slice (e on partitions)
            wpt = cst.tile([128, 2, E], F32R)          # W_proj slice (c on partitions)
            mk = cst.tile([128, 128], F32R)            # tril(128) causal mask

            # weights first, then x chunk 0, so QKV of chunk 0 starts ASAP;
            # remaining x chunks stream in behind it
            wr = w[:].rearrange("(t p) j -> p t j", p=128)
            xTr = xT[:].rearrange("(t p) s -> p t s", p=128)
            nc.sync.dma_start(wt[:, 0:4, 0:512], wr[:, 0:4, 0:512])
            nc.sync.dma_start(xt[:, 0:4, 0:512], xTr[:, 0:4, 0:512])
            nc.sync.dma_start(wt[:, 4:8, 0:512], wr[:, 4:8, 0:512])
            nc.sync.dma_start(xt[:, 4:8, 0:512], xTr[:, 4:8, 0:512])
            nc.sync.dma_start(wt[:, :, 512:768], wr[:, :, 512:768])
            nc.sync.dma_start(mk[:], mask[:])
            nc.sync.dma_start(wpt[:], wp[:].rearrange("(t p) e -> p t e", p=128))
            for sc in range(1, NS):
                nc.sync.dma_start(
                    xt[:, :, 512 * sc:512 * (sc + 1)], xTr[:, :, 512 * sc:512 * (sc + 1)]
                )

            # ---- activation buffers ---------------------------------------
            # qt/kt: [pair, j(128 part: head 2p on 0-63, head 2p+1 on 64-127), s]
            qt = acts.tile([128, 2, S], F32R)
            kt = acts.tile([128, 2, S], F32R)
            # v_aug: per s-tile, per head 65 cols (64 data + ones)
            vt = acts.tile([128, NT, HL * 65], F32R)
            # attention output^T, proj lhsT layout: c on partitions
            ot = acts.tile([128, 2, S], F32R)

            nc.vector.memset(vt[:].bitcast(F32), 1.0)
            warm = cst.tile([128, 16], F32)
            nc.scalar.activation(warm[:], vt[:, 0, 0:16].bitcast(F32),
                                 mybir.ActivationFunctionType.Exp)
            # dummy matmuls during the DMA head keep the PE HAM-warm so QKV
            # starts at full clock
            vflat = vt[:].rearrange("p a b -> p (a b)")
            for wi in range(22):
                pw = psS.tile([128, 1024], F32, tag="ps", name="pw")
                nc.tensor.matmul(pw[:, 0:512], vflat[:, 0:128], vflat[:, 512:1024],
                                 start=True, stop=True)

            # ---- QKV projection ------------------------------------------
            def qk_pair(sc, jt):
                s0 = 512 * sc
                for which, dest in ((0, qt), (1, kt)):
                    ps = psO.tile([128, 512], F32, tag="po", name="psqk")
                    for et in range(ET):
                        nc.tensor.matmul(
                            ps[:, 0:512],
                            wt[:, et, 256 * which + 128 * jt:256 * which + 128 * (jt + 1)],
                            xt[:, et, s0:s0 + 512],
                            start=(et == 0),
                            stop=(et == ET - 1),
                        )
                    nc.vector.tensor_copy(dest[:, jt, s0:s0 + 512], ps[:, 0:512])

            def v_chunk(sc, lo=0, hi=4):
                for st4 in range(lo, hi):
                    st = 4 * sc + st4
                    ps = psO.tile([128, 512], F32, tag="po", name="psv")
                    for et in range(ET):
                        nc.tensor.matmul(
                            ps[:, 0:256],
                            xt[:, et, 128 * st:128 * (st + 1)],
                            wt[:, et, 512:768],
                            start=(et == 0),
                            stop=(et == ET - 1),
                        )
                    nc.vector.tensor_copy(
                        vt[:, st].rearrange("p (h m) -> p h m", h=HL)[:, :, 0:64],
                        ps[:, 0:256].rearrange("p (h m) -> p h m", h=HL),
                    )

            # ---- attention for one (pair, q-chunk) ------------------------
            def attn_part(pr, jq, ik_lo, ik_hi, o_ab):
                s0 = 512 * jq
                nik = 4 * jq + 4
                for ik in range(ik_lo, ik_hi):
                    t = ik - 4 * jq
                    c0 = 128 * t if t > 0 else 0   # exact-causal column trim
                    ps = psS.tile([128, 1024], F32)
                    for ab in range(2):
                        p0 = 64 * ab
                        nc.tensor.matmul(
                            ps[:, 512 * ab + c0:512 * (ab + 1)],
                            kt[p0:p0 + 64, pr, 128 * ik:128 * (ik + 1)],
                            qt[p0:p0 + 64, pr, s0 + c0:s0 + 512],
                            start=True,
                            stop=True,
                            tile_position=(p0, 0),
                        )
                    e = expp.tile([128, 1024], F32R, tag="exps", name="exps")
                    e3 = e[:].rearrange("p (h n) -> p h n", h=2)[:, :, c0:512]
                    ps3 = ps[:].rearrange("p (h n) -> p h n", h=2)[:, :, c0:512]
                    nc.scalar.activation(e3, ps3, mybir.ActivationFunctionType.Exp,
                                         scale=float(SCALE))
                    if t >= 0:
                        # only the first 128 cols of the trimmed range are
                        # partially masked; the rest is fully unmasked
                        for ab in range(2):
                            nc.vector.tensor_mul(
                                e[:, 512 * ab + c0:512 * ab + c0 + 128],
                                e[:, 512 * ab + c0:512 * ab + c0 + 128],
                                mk[:],
                            )
                    for ab in range(2):
                        h = 2 * pr + ab
                        nc.tensor.matmul(
                            o_ab[ab][0:65, c0:512],
                            vt[:, ik, 65 * h:65 * h + 65],
                            e[:, 512 * ab + c0:512 * (ab + 1)],
                            start=(ik == 0),
                            stop=(ik == nik - 1),
                            skip_group_check=True,
                        )
            # normalize: out^T[d, s] = o[d, s] * (1 / o[64, s]).
            # Engines can shift partitions between in and out APs, so head
            # ab=1 writes ot partitions 64-127 directly.
            def attn_norm(pr, jq, o_ab):
                s0 = 512 * jq
                for ab in range(2):
                    o = o_ab[ab]
                    osb = small.tile([128, 512], F32R, tag="osb", name="osb")
                    nc.scalar.copy(osb[0:65, :], o[0:65, :])
                    # r row to physical partition 0 (partition_broadcast needs it)
                    rz = small.tile([128, 512], F32, tag="rz", name="rz")
                    nc.vector.tensor_copy(rz[0:1, :], osb[64:65, :].bitcast(F32))
                    rb = small.tile([128, 512], F32, tag="rb", name="rb")
                    nc.gpsimd.partition_broadcast(rb[0:64, :], rz[0:1, :])
                    rinv = small.tile([128, 512], F32, tag="rinv", name="rinv")
                    nc.vector.reciprocal_approx_fast(rinv[0:64, :], rb[0:64, :])
                    nc.vector.tensor_mul(ot[64 * ab:64 * ab + 64, pr, s0:s0 + 512],
                                         osb[0:64, :], rinv[0:64, :].bitcast(F32R))

            # ---- output projection for one s-chunk ------------------------
            def proj(jq, copy_eng=None):
                for st4 in range(4):
                    st = 4 * jq + st4
                    for nk in range(2):
                        py = psO.tile([128, 512], F32, tag="po", name="py")
                        for ct in range(2):
                            nc.tensor.matmul(
                                py[:],
                                ot[:, ct, 128 * st:128 * (st + 1)],
                                wpt[:, ct, 512 * nk:512 * (nk + 1)],
                                start=(ct == 0),
                                stop=(ct == 1),
                            )
                        ys = ysbp.tile([128, 512], F32)
                        if copy_eng == "scalar":
                            nc.scalar.copy(ys[:], py[:])
                        else:
                            nc.vector.tensor_copy(ys[:], py[:])
                        nc.sync.dma_start(
                            y[128 * st:128 * (st + 1), 512 * nk:512 * (nk + 1)], ys[:]
                        )

            # Interleave QKV chunks and proj between attention sub-phases:
            # attention is ACT(exp)-bound, so the PE queue gets dense
            # independent matmul work to stay HAM-warm, and proj trails one
            # sub-phase behind so its ot deps (incl. the head-B shift DMA)
            # are long complete when the PE reaches it.
            def attn_full(pr, jq, mids):
                """attention with qkv/proj work interleaved at explicit
                ik split points: mids = [(split_ik, fn), ...] ascending"""
                nik = 4 * jq + 4
                o_ab = [psO.tile([128, 512], F32, tag="po", name="o_ab")
                        for _ in range(2)]
                prev = 0
                for split, fn in mids:
                    attn_part(pr, jq, prev, split, o_ab)
                    fn()
                    prev = split
                attn_part(pr, jq, prev, nik, o_ab)
                attn_norm(pr, jq, o_ab)

            qk_pair(0, 0)
            v_chunk(0)
            attn_full(0, 0, [(2, lambda: qk_pair(0, 1))])
            attn_full(1, 0, [(2, lambda: qk_pair(1, 0)),
                             (2, lambda: qk_pair(1, 1))])
            attn_full(0, 1, [(4, lambda: v_chunk(1))])
            proj(0)
            attn_full(1, 1, [(4, lambda: qk_pair(2, 0))])
            attn_full(0, 2, [(6, lambda: qk_pair(2, 1)),
                             (6, lambda: v_chunk(2))])
            proj(1)
            attn_full(1, 2, [(6, lambda: qk_pair(3, 0))])
            attn_full(0, 3, [(8, lambda: qk_pair(3, 1)),
                             (8, lambda: v_chunk(3))])
            proj(2)
            attn_full(1, 3, [])
            proj(3, copy_eng="scalar")

    nc.compile()
    return nc


def _get_nc():
    global _CACHED_NC
    if _CACHED_NC is None:
        _CACHED_NC = _build()
    return _CACHED_NC


def _diag_masks() -> np.ndarray:
    return np.ascontiguousarray(np.tril(np.ones((128, 128), dtype=np.float32)).T)


def _in_maps(x, W_qkv, W_proj):
    masks = _diag_masks()
    maps = []
    for c in range(N_CORES):
        b, g = divmod(c, 4)
        xT = np.ascontiguousarray(x[b].T.astype(np.float32))
        wq = W_qkv[:, GC * g:GC * (g + 1)]
        wk = W_qkv[:, E + GC * g:E + GC * (g + 1)]
        wv = W_qkv[:, 2 * E + GC * g:2 * E + GC * (g + 1)]
        w = np.ascontiguousarray(
            np.concatenate([wq, wk, wv], axis=1).astype(np.float32))
        wp = np.ascontiguousarray(W_proj[GC * g:GC * (g + 1), :].astype(np.float32))
        maps.append({"xT": xT, "w": w, "wp": wp, "mask": masks})
    return maps


def _run(x, W_qkv, W_proj, trace=False, **spmd_kwargs):
    nc = _get_nc()
    res = run_bass_kernel_spmd(nc, _in_maps(x, W_qkv, W_proj),
                               list(range(N_CORES)), trace=trace, **spmd_kwargs)
    out = np.zeros((B, S, E), dtype=np.float32)
    for c in range(N_CORES):
        out[c // 4] += res.results[c]["y"]
    return out, res


def kernel(x, attention_mask, W_qkv, W_proj):
    x = np.asarray(x, dtype=np.float32)
    W_qkv = np.asarray(W_qkv, dtype=np.float32)
    W_proj = np.asarray(W_proj, dtype=np.float32)
    out, _ = _run(x, W_qkv, W_proj, trace=False)
    return out
